# revision 15
# baseline (speedup 1.0000x reference)
"""Trainium2 Bass kernel for nn_DistanceNetwork (retrieval_knn).

out[b, s, j] = dot[s, j] / (||sup[s, b]|| * ||inp[b]|| + EPS)
  dot[s, j] = sum_d sup[s, j, d] * inp[j, d]

Sharding: S=8192 split across 8 cores (1024 each). The host casts the
support slice to bf16 and pre-transposes it to [TILES, D, (b s)] so each
128-s tile lands in SBUF as supT[d, b*128+s] via plain contiguous DMA.
rel-err budget: bf16 rounding ~0.2-0.4% of max vs tolerance 2e-2.

Per 128-s tile:
 - PE:  32 matvecs (lhsT=supT_b [d,s], rhs=inpT[:,b]) -> psum dot[s,b];
        32 matvecs (lhsT=sqT_b,       rhs=ones)       -> psum norm2[s,b].
   (bf16 ldweights+FD=1 matmul pairs pipeline at ~27ns each.)
 - DVE: squares for b<K_SQ_DVE (bf16 2x packed); den = sn*tnorm;
        rden = 1/den; outer product outt = rden x dot -> bf16.
 - ACT: squares for b>=K_SQ_DVE; psum->sbuf dot copy; sn = sqrt(norm2).
 - DMA: tile loads paired (2 MiB transfers) alternating sync/scalar
   HWDGE rings; stores via gpsimd SWDGE. Output written s-major
   [SL, (b j)] bf16; the host transposes/casts to [B, S, B] f32.
"""

import os
import sys

import numpy as np
import ml_dtypes

for _p in ("/opt/trn_rl_repo", "/root/.axon_site/_ro/trn_rl_repo"):
    if os.path.isdir(_p) and _p not in sys.path:
        sys.path.insert(0, _p)

import concourse.bass as bass
import concourse.bacc as bacc
import concourse.mybir as mybir
from concourse.bass_utils import run_bass_kernel_spmd
from concourse.tile import TileContext

S, B, D = 8192, 32, 128
NCORES = 8
SL = S // NCORES          # 1024 s-rows per core
P = 128                   # partition tile of s
TILES = SL // P           # 8 s-tiles per core
BP = B * P                # 4096 free elems per supT tile
F32 = mybir.dt.float32
BF16 = mybir.dt.bfloat16
K_SQ_DVE = 13             # b-segments squared on DVE; the rest on ACT


def _build_nc():
    nc = bacc.Bacc()
    supT = nc.declare_dram_parameter("supT", [TILES, D, BP], BF16, isOutput=False)
    inpT = nc.declare_dram_parameter("inpT", [D, B], BF16, isOutput=False)
    tnh = nc.declare_dram_parameter("tnorm", [1, B], F32, isOutput=False)
    out = nc.declare_dram_parameter("out", [SL, B * B], BF16, isOutput=True)
    Sqrt = mybir.ActivationFunctionType.Sqrt
    SQUARE = mybir.ActivationFunctionType.Square

    # load schedule: tiles 0,1 single (fast pipeline fill), then pairs
    chunks = [(0, 1), (1, 1), (2, 2), (4, 2), (6, 2)]

    with TileContext(nc) as tc:
        with (
            tc.tile_pool(name="psum", bufs=3, space="PSUM") as ppool,
            tc.tile_pool(name="const", bufs=1) as cpool,
            tc.tile_pool(name="sup", bufs=5) as suppool,
            tc.tile_pool(name="sq", bufs=3) as sqpool,
            tc.tile_pool(name="outp", bufs=3) as opool,
            tc.tile_pool(name="small", bufs=3) as spool,
        ):
            ones = cpool.tile([D, 1], BF16)
            nc.gpsimd.memset(ones[:], 1.0)
            inp_sb = cpool.tile([D, B], BF16)
            tnorm = cpool.tile([P, B], F32)
            nc.scalar.dma_start(out=inp_sb[:], in_=inpT[:, :])
            nc.scalar.dma_start(out=tnorm[:], in_=tnh[:, :].broadcast_to([P, B]))
            # trigger the Sqrt ACT table load off the critical path
            warm = cpool.tile([P, B], F32)
            nc.scalar.activation(warm[:], tnorm[:], Sqrt)

            for ci, (t0, ntile) in enumerate(chunks):
                sup_c = suppool.tile([D, ntile * BP], BF16, tag="sup")
                eng = nc.sync if ci % 2 == 0 else nc.scalar
                eng.dma_start(
                    out=sup_c[:].rearrange("d (t f) -> d t f", t=ntile),
                    in_=supT[t0 : t0 + ntile, :, :].rearrange("t d f -> d t f"),
                )
                for ti in range(ntile):
                    t = t0 + ti
                    sup_t = sup_c[:, ti * BP : (ti + 1) * BP]

                    # squares for the norms: split DVE (2x packed) / ACT.
                    # tile 0: all on DVE — ACT's half would sit in the
                    # first tile's critical chain during pipeline fill.
                    sq_t = sqpool.tile([D, BP], BF16, tag="sq")
                    KD = (32 if t == 0 else K_SQ_DVE) * P
                    nc.vector.tensor_mul(
                        sq_t[:, 0:KD], sup_t[:, 0:KD], sup_t[:, 0:KD]
                    )
                    if KD < BP:
                        nc.scalar.activation(
                            sq_t[:, KD:BP], sup_t[:, KD:BP], SQUARE
                        )

                    # PE matvecs: dot[s, b] then norm2[s, b], both [P, B] f32
                    ps = ppool.tile([P, 2 * B], F32, tag="ps")
                    for b in range(B):
                        nc.tensor.matmul(
                            ps[:, b : b + 1],
                            sup_t[:, b * P : (b + 1) * P],
                            inp_sb[:, b : b + 1],
                            start=True,
                            stop=True,
                        )
                    for b in range(B):
                        nc.tensor.matmul(
                            ps[:, B + b : B + b + 1],
                            sq_t[:, b * P : (b + 1) * P],
                            ones[:, 0:1],
                            start=True,
                            stop=True,
                        )

                    # sqrt, denominator, reciprocal
                    sn = spool.tile([P, B], F32, tag="sn")
                    nc.scalar.activation(sn[:], ps[:, B : 2 * B], Sqrt)
                    den = spool.tile([P, B], F32, tag="den")
                    nc.vector.tensor_mul(den[:], sn[:], tnorm[:])
                    rden = spool.tile([P, B], F32, tag="rden")
                    nc.vector.reciprocal_approx_fast(rden[:], den[:])

                    # outer product -> bf16; dot is read straight from PSUM
                    outt = opool.tile([P, B * B], BF16, tag="outt")
                    nc.vector.tensor_mul(
                        outt[:].rearrange("p (b j) -> p b j", j=B),
                        rden[:].unsqueeze(2).broadcast_to([P, B, B]),
                        ps[:, 0:B].unsqueeze(1).broadcast_to([P, B, B]),
                    )
                    nc.gpsimd.dma_start(
                        out=out[t * P : (t + 1) * P, :], in_=outt[:]
                    )
    if not nc.is_finalized():
        nc.finalize()
    return nc


_NC = None
last_results = None


def _get_nc():
    global _NC
    if _NC is None:
        _NC = _build_nc()
    return _NC


def kernel(support_set: np.ndarray, input_signal: np.ndarray) -> np.ndarray:
    global last_results
    nc = _get_nc()

    inp32 = np.ascontiguousarray(input_signal, dtype=np.float32)
    sup_bf = np.asarray(support_set, dtype=np.float32).astype(ml_dtypes.bfloat16)
    inpT = np.ascontiguousarray(inp32.T.astype(ml_dtypes.bfloat16))
    tnorm = np.sqrt(np.sum(inp32 * inp32, axis=1)).reshape(1, B)
    tnorm = np.ascontiguousarray(tnorm, dtype=np.float32)

    in_maps = []
    for i in range(NCORES):
        sl = sup_bf[i * SL : (i + 1) * SL]            # [SL, B, D]
        st = sl.reshape(TILES, P, B, D).transpose(0, 3, 2, 1)  # [t, d, b, s]
        in_maps.append(
            {
                "supT": np.ascontiguousarray(st.reshape(TILES, D, BP)),
                "inpT": inpT,
                "tnorm": tnorm,
            }
        )

    res = run_bass_kernel_spmd(nc, in_maps, list(range(NCORES)))
    last_results = res

    final = np.empty((B, S, B), dtype=np.float32)
    for i in range(NCORES):
        o = np.asarray(res.results[i]["out"]).reshape(SL, B, B)
        final[:, i * SL : (i + 1) * SL, :] = o.transpose(1, 0, 2)
    return final
